# revision 14
# baseline (speedup 1.0000x reference)
"""KoLeo-loss kernel for Trainium2 (Bass/Tile), data-parallel over batch on 8 cores.

Input : student_output [8, 4096, 256] fp32
Output: scalar fp32 loss = -mean(log(||x - x_nn + 1e-8||_2 + 1e-8))
        where x_nn[b,t] = x[b, argmax_s <x[b,t], x[b,s]> (diag excluded)].

Per-core plan (core b handles batch b), "S7-fp8" scheme:
  - PE: gram matrix dots = x @ x.T in 32 m-tiles of [128, 4096] using
        fp8e4m3 inputs with DoubleRow perf mode: one matmul contracts
        the full K=256 (two 128-deep planes packed per PE cell) at 0.5
        cycles/row.  fp8 quantization only perturbs the *selection* of
        the argmax (ties flip to a near-equal neighbor); distances are
        computed from bf16 rows, so the loss error stays ~2e-4.
  - ACT: PSUM -> SBUF copies, downcast to bf16.
  - DVE: pairwise-max fold cascade 4096 -> 2048 -> ... -> 128 (2x mode
        in bf16), MAX8 on the 128-wide tail (top-1 is always the
        diagonal; top-2 is the NN dot), FIND_INDEX8 on the 2048-wide
        fold level.  The NN's true column is idx or idx+2048: both are
        evaluated and the row's distance is their min (flips ~0.6% of
        rows to a closer-but-lower-dot neighbor; ~2e-4 rel loss impact).
  - GPSIMD/DGE: one combined gather of both candidate rows (bf16).
  - PE: diff_k = x_t - cand_k via identity-matmul accumulation.
  - ACT: dist2_k = sum((diff_k + 1e-8)^2) via Square with accumulate.
  - host: diagonal guard + min over the 2 candidates, then
        loss = -mean(log(sqrt(dist2) + 1e-8)) in f64, over all 8 cores.
"""

import numpy as np
import ml_dtypes

import concourse.bass as bass
import concourse.tile as tile
from concourse import bacc, mybir
from concourse import bass_utils

F32 = mybir.dt.float32
BF16 = mybir.dt.bfloat16
FP8 = mybir.dt.float8e4
U32 = mybir.dt.uint32

B, T, D = 8, 4096, 256
P = 128                  # partitions
M = T // P               # 32 m-tiles
KC = D // P              # 2 contraction planes (DoubleRow)
EPS = 1e-8


def build_bass(num_devices=8):
    nc = bacc.Bacc("TRN2", target_bir_lowering=False, debug=False,
                   num_devices=num_devices)
    xT8 = nc.dram_tensor("xT8", [P, KC, T], FP8, kind="ExternalInput")
    xrb = nc.dram_tensor("xrb", [P, M * D], BF16, kind="ExternalInput")
    xgb = nc.dram_tensor("xgb", [T, D], BF16, kind="ExternalInput")
    ident = nc.dram_tensor("ident", [P, 2 * P], BF16, kind="ExternalInput")
    d2_out = nc.dram_tensor("d2", [P, 2 * M], F32, kind="ExternalOutput")
    off_out = nc.dram_tensor("off", [P, 2 * M], U32, kind="ExternalOutput")

    with tile.TileContext(nc) as tc:
        with (
            tc.tile_pool(name="const", bufs=1) as const_pool,
            tc.tile_pool(name="dots", bufs=2) as dots_pool,
            tc.tile_pool(name="w2048", bufs=2) as w2048_pool,
            tc.tile_pool(name="cands", bufs=3) as cands_pool,
            tc.tile_pool(name="pdots", bufs=3, space="PSUM") as pdots_pool,
            tc.tile_pool(name="pdiff", bufs=2, space="PSUM") as pdiff_pool,
            tc.tile_pool(name="small", bufs=4) as small_pool,
            tc.tile_pool(name="res", bufs=1) as res_pool,
        ):
            # resident inputs.  xT8 is loaded in 4 column chunks, issued
            # first, so the first matmuls only wait on chunk 0 (~256 KB)
            # instead of the full 5 MB input load (~13 us of startup).
            xT8_c = [const_pool.tile([P, KC, 1024], FP8, name=f"xT8c{q}",
                                     tag=f"xT8c{q}") for q in range(4)]
            for q in range(4):
                nc.sync.dma_start(xT8_c[q][:], xT8[:, :, 1024 * q:1024 * (q + 1)])
            xr_sb = const_pool.tile([P, M * D], BF16, tag="xr")
            nc.sync.dma_start(xr_sb[:], xrb[:])
            id_sb = const_pool.tile([P, 2 * P], BF16, tag="ident")
            nc.sync.dma_start(id_sb[:], ident[:])
            epsb = const_pool.tile([P, 1], F32, tag="epsb")
            nc.vector.memset(epsb[:], EPS)

            d2_all = res_pool.tile([P, 2 * M], F32, tag="d2")
            off_all = res_pool.tile([P, 2 * M], U32, tag="off")

            dots_t = [None] * M   # bf16 dots tiles
            cand_t = [None] * M   # gathered candidate rows
            pd_t = [None] * M     # diff PSUM tiles

            def stage_a(m):
                # PE: dots in 4 PSUM quarters (2 DoubleRow mms each);
                # ACT: copy to bf16
                dots = dots_pool.tile([P, T], BF16, tag="dots")
                dots_t[m] = dots
                lhsT = xT8_c[m // 8][:, :, (m % 8) * P:((m % 8) + 1) * P]
                for q in range(4):
                    ps = pdots_pool.tile([P, 1024], F32, tag="ps")
                    for jj in range(2):
                        nc.tensor.matmul(
                            ps[:, jj * 512:(jj + 1) * 512],
                            lhsT=lhsT,
                            rhs=xT8_c[q][:, :, jj * 512:(jj + 1) * 512],
                            start=True, stop=True,
                            perf_mode=mybir.MatmulPerfMode.DoubleRow)
                    nc.scalar.copy(dots[:, q * 1024:(q + 1) * 1024], ps[:])

            def stage_b(m):
                # DVE: fold-max cascade + MAX8 + FIND_INDEX8@2048 + offsets
                dots = dots_t[m]
                w2048 = w2048_pool.tile([P, 2048], BF16, tag="w2048")
                nc.vector.tensor_tensor(
                    out=w2048[:], in0=dots[:, 0:2048], in1=dots[:, 2048:4096],
                    op=mybir.AluOpType.max)
                prev = w2048
                width = 1024
                folds = {}
                while width >= 128:
                    wt = small_pool.tile([P, width], BF16, tag=f"w{width}")
                    nc.vector.tensor_tensor(
                        out=wt[:], in0=prev[:, 0:width], in1=prev[:, width:2 * width],
                        op=mybir.AluOpType.max)
                    folds[width] = wt
                    prev = wt
                    width //= 2
                top8 = small_pool.tile([P, 8], BF16, tag="top8")
                nc.vector.max(out=top8[:], in_=folds[128][:])
                idx8 = small_pool.tile([P, 8], U32, tag="idx8")
                nc.vector.max_index(out=idx8[:], in_max=top8[:], in_values=w2048[:])

                # candidate columns: idx2 (clamped) and idx2 + 2048,
                # written straight into the resident output tile
                offs = off_all[:, 2 * m:2 * m + 2]
                nc.vector.tensor_scalar(
                    out=offs[:, 0:1], in0=idx8[:, 1:2], scalar1=2047,
                    scalar2=None, op0=mybir.AluOpType.min)
                nc.vector.tensor_scalar(
                    out=offs[:, 1:2], in0=offs[:, 0:1], scalar1=2048,
                    scalar2=None, op0=mybir.AluOpType.add)

            def stage_c(m):
                # GPSIMD/DGE: gather the two candidate rows (one indirect
                # DMA per offset column; a combined [P,2]-offset gather
                # returns garbage for the second slot on this runtime)
                cands = cands_pool.tile([P, 2 * D], BF16, tag="cands")
                cand_t[m] = cands
                for k in range(2):
                    nc.gpsimd.indirect_dma_start(
                        out=cands[:, k * D:(k + 1) * D], out_offset=None,
                        in_=xgb[:],
                        in_offset=bass.IndirectOffsetOnAxis(
                            ap=off_all[:, 2 * m + k:2 * m + k + 1], axis=0))

            def stage_d(m):
                # PE: diff_k = x_t - cand_k via identity matmuls
                pd = pdiff_pool.tile([P, 2 * D], F32, tag="pd")
                pd_t[m] = pd
                for k in range(2):
                    nc.tensor.matmul(
                        pd[:, k * D:(k + 1) * D],
                        lhsT=id_sb[:, 0:P],
                        rhs=xr_sb[:, m * D:(m + 1) * D],
                        start=True, stop=False)
                    nc.tensor.matmul(
                        pd[:, k * D:(k + 1) * D],
                        lhsT=id_sb[:, P:2 * P],
                        rhs=cand_t[m][:, k * D:(k + 1) * D],
                        start=False, stop=True)

            def stage_e(m):
                # ACT: dist2_k = sum((diff_k + eps)^2) -> resident tile
                pd = pd_t[m]
                sq = small_pool.tile([P, 2 * D], BF16, tag="sq")
                for k in range(2):
                    nc.scalar.activation(
                        out=sq[:, k * D:(k + 1) * D], in_=pd[:, k * D:(k + 1) * D],
                        func=mybir.ActivationFunctionType.Square,
                        bias=epsb[:], scale=1.0,
                        accum_out=d2_all[:, 2 * m + k:2 * m + k + 1])

            for m in range(M + 2):
                if 2 <= m <= M + 1:
                    stage_d(m - 2)
                    stage_e(m - 2)
                if m < M:
                    stage_a(m)
                if 1 <= m <= M:
                    stage_b(m - 1)
                    stage_c(m - 1)

            nc.sync.dma_start(d2_out[:], d2_all[:])
            nc.sync.dma_start(off_out[:], off_all[:])
    nc.compile()
    return nc


_CACHE = {}


def _built():
    if "nc" not in _CACHE:
        _CACHE["nc"] = build_bass(8)
    return _CACHE["nc"]


def make_in_maps(x):
    x = np.ascontiguousarray(np.asarray(x, dtype=np.float32))
    assert x.shape == (B, T, D)
    idm = np.eye(P, dtype=ml_dtypes.bfloat16)
    ident = np.ascontiguousarray(np.concatenate([idm, -idm], axis=1))
    in_maps = []
    for b in range(B):
        xb = x[b].astype(ml_dtypes.bfloat16)
        x8 = x[b].astype(ml_dtypes.float8_e4m3)
        # xT8[ki, ko, t] = x8[t, ko*128 + ki]
        xT8 = np.ascontiguousarray(
            x8.T.reshape(KC, P, T).transpose(1, 0, 2))
        in_maps.append({
            "xT8": xT8,
            "xrb": np.ascontiguousarray(
                xb.reshape(M, P, D).transpose(1, 0, 2)).reshape(P, M * D),
            "xgb": xb,
            "ident": ident,
        })
    return in_maps


def postprocess(results):
    """results: per-core dicts with d2 [128, 2M] and off [128, 2M].
    Row t = 128*m + p holds candidates at columns (2m, 2m+1)."""
    total = 0.0
    n = 0
    for r in results:
        d2 = r["d2"].astype(np.float64).reshape(P, M, 2)
        off = r["off"].astype(np.int64).reshape(P, M, 2)
        rowid = np.arange(P)[:, None, None] + 128 * np.arange(M)[None, :, None]
        d2 = np.where(off == rowid, np.inf, d2).min(axis=2)  # diag guard + min
        d = np.sqrt(d2)
        total += np.log(d + EPS).sum()
        n += d.size
    return np.float32(-(total / n))


def kernel(student_output):
    nc = _built()
    in_maps = make_in_maps(student_output)
    res = bass_utils.run_bass_kernel_spmd(nc, in_maps, core_ids=list(range(B)))
    return postprocess([res.results[b] for b in range(B)])


# revision 16
# speedup vs baseline: 1.0220x; 1.0220x over previous
"""KoLeo-loss kernel for Trainium2 (Bass/Tile), data-parallel over batch on 8 cores.

Input : student_output [8, 4096, 256] fp32
Output: scalar fp32 loss = -mean(log(||x - x_nn + 1e-8||_2 + 1e-8))
        where x_nn[b,t] = x[b, argmax_s <x[b,t], x[b,s]> (diag excluded)].

Per-core plan (core b handles batch b), "S7-fp8" scheme:
  - PE: gram matrix dots = x @ x.T in 32 m-tiles of [128, 4096] using
        fp8e4m3 inputs with DoubleRow perf mode: one matmul contracts
        the full K=256 (two 128-deep planes packed per PE cell) at 0.5
        cycles/row.  fp8 quantization only perturbs the *selection* of
        the argmax (ties flip to a near-equal neighbor); distances are
        computed from bf16 rows, so the loss error stays ~2e-4.
  - ACT: PSUM -> SBUF copies, downcast to bf16.
  - DVE: pairwise-max fold cascade 4096 -> 2048 -> ... -> 128 (2x mode
        in bf16), MAX8 on the 128-wide tail (top-1 is always the
        diagonal; top-2 is the NN dot), FIND_INDEX8 on the 2048-wide
        fold level.  The NN's true column is idx or idx+2048: both are
        evaluated and the row's distance is their min (flips ~0.6% of
        rows to a closer-but-lower-dot neighbor; ~2e-4 rel loss impact).
  - GPSIMD/DGE: one combined gather of both candidate rows (bf16).
  - PE: diff_k = x_t - cand_k via identity-matmul accumulation.
  - ACT: dist2_k = sum((diff_k + 1e-8)^2) via Square with accumulate.
  - host: diagonal guard + min over the 2 candidates, then
        loss = -mean(log(sqrt(dist2) + 1e-8)) in f64, over all 8 cores.
"""

import numpy as np
import ml_dtypes

import concourse.bass as bass
import concourse.tile as tile
from concourse import bacc, mybir
from concourse import bass_utils

F32 = mybir.dt.float32
BF16 = mybir.dt.bfloat16
FP8 = mybir.dt.float8e4
U32 = mybir.dt.uint32

B, T, D = 8, 4096, 256
P = 128                  # partitions
M = T // P               # 32 m-tiles
KC = D // P              # 2 contraction planes (DoubleRow)
EPS = 1e-8


def build_bass(num_devices=8):
    nc = bacc.Bacc("TRN2", target_bir_lowering=False, debug=False,
                   num_devices=num_devices)
    xT8 = nc.dram_tensor("xT8", [P, KC, T], FP8, kind="ExternalInput")
    xrb = nc.dram_tensor("xrb", [P, M * D], BF16, kind="ExternalInput")
    xgb = nc.dram_tensor("xgb", [T, D], BF16, kind="ExternalInput")
    ident = nc.dram_tensor("ident", [P, 2 * P], BF16, kind="ExternalInput")
    d2_out = nc.dram_tensor("d2", [P, 2 * M], F32, kind="ExternalOutput")
    off_out = nc.dram_tensor("off", [P, 2 * M], U32, kind="ExternalOutput")

    with tile.TileContext(nc) as tc:
        with (
            tc.tile_pool(name="const", bufs=1) as const_pool,
            tc.tile_pool(name="dots", bufs=2) as dots_pool,
            tc.tile_pool(name="w2048", bufs=2) as w2048_pool,
            tc.tile_pool(name="cands", bufs=3) as cands_pool,
            tc.tile_pool(name="pdots", bufs=3, space="PSUM") as pdots_pool,
            tc.tile_pool(name="pdiff", bufs=2, space="PSUM") as pdiff_pool,
            tc.tile_pool(name="small", bufs=4) as small_pool,
            tc.tile_pool(name="res", bufs=1) as res_pool,
        ):
            # resident inputs
            xT8_sb = const_pool.tile([P, KC, T], FP8, tag="xT8")
            nc.sync.dma_start(xT8_sb[:], xT8[:])
            xr_sb = const_pool.tile([P, M * D], BF16, tag="xr")
            nc.sync.dma_start(xr_sb[:], xrb[:])
            id_sb = const_pool.tile([P, 2 * P], BF16, tag="ident")
            nc.sync.dma_start(id_sb[:], ident[:])
            epsb = const_pool.tile([P, 1], F32, tag="epsb")
            nc.vector.memset(epsb[:], EPS)

            d2_all = res_pool.tile([P, 2 * M], F32, tag="d2")
            off_all = res_pool.tile([P, 2 * M], U32, tag="off")

            dots_t = [None] * M   # bf16 dots tiles
            cand_t = [None] * M   # gathered candidate rows
            pd_t = [None] * M     # diff PSUM tiles

            def stage_a(m):
                # PE: dots in 4 PSUM quarters (2 DoubleRow mms each);
                # ACT: copy to bf16
                dots = dots_pool.tile([P, T], BF16, tag="dots")
                dots_t[m] = dots
                lhsT = xT8_sb[:, :, m * P:(m + 1) * P]
                for q in range(4):
                    ps = pdots_pool.tile([P, 1024], F32, tag="ps")
                    for jj in range(2):
                        j0 = q * 1024 + jj * 512
                        nc.tensor.matmul(
                            ps[:, jj * 512:(jj + 1) * 512],
                            lhsT=lhsT,
                            rhs=xT8_sb[:, :, j0:j0 + 512],
                            start=True, stop=True,
                            perf_mode=mybir.MatmulPerfMode.DoubleRow)
                    nc.scalar.copy(dots[:, q * 1024:(q + 1) * 1024], ps[:])

            def stage_b(m):
                # DVE: fold-max cascade + MAX8 + FIND_INDEX8@2048 + offsets
                dots = dots_t[m]
                w2048 = w2048_pool.tile([P, 2048], BF16, tag="w2048")
                nc.vector.tensor_tensor(
                    out=w2048[:], in0=dots[:, 0:2048], in1=dots[:, 2048:4096],
                    op=mybir.AluOpType.max)
                prev = w2048
                width = 1024
                folds = {}
                while width >= 128:
                    wt = small_pool.tile([P, width], BF16, tag=f"w{width}")
                    nc.vector.tensor_tensor(
                        out=wt[:], in0=prev[:, 0:width], in1=prev[:, width:2 * width],
                        op=mybir.AluOpType.max)
                    folds[width] = wt
                    prev = wt
                    width //= 2
                top8 = small_pool.tile([P, 8], BF16, tag="top8")
                nc.vector.max(out=top8[:], in_=folds[128][:])
                idx8 = small_pool.tile([P, 8], U32, tag="idx8")
                nc.vector.max_index(out=idx8[:], in_max=top8[:], in_values=w2048[:])

                # candidate columns: idx2 (clamped) and idx2 + 2048,
                # written straight into the resident output tile
                offs = off_all[:, 2 * m:2 * m + 2]
                nc.vector.tensor_scalar(
                    out=offs[:, 0:1], in0=idx8[:, 1:2], scalar1=2047,
                    scalar2=None, op0=mybir.AluOpType.min)
                nc.vector.tensor_scalar(
                    out=offs[:, 1:2], in0=offs[:, 0:1], scalar1=2048,
                    scalar2=None, op0=mybir.AluOpType.add)

            def stage_c(m):
                # GPSIMD/DGE: gather the two candidate rows (one indirect
                # DMA per offset column; a combined [P,2]-offset gather
                # returns garbage for the second slot on this runtime)
                cands = cands_pool.tile([P, 2 * D], BF16, tag="cands")
                cand_t[m] = cands
                for k in range(2):
                    nc.gpsimd.indirect_dma_start(
                        out=cands[:, k * D:(k + 1) * D], out_offset=None,
                        in_=xgb[:],
                        in_offset=bass.IndirectOffsetOnAxis(
                            ap=off_all[:, 2 * m + k:2 * m + k + 1], axis=0))

            def stage_d(m):
                # PE: diff_k = x_t - cand_k via identity matmuls
                pd = pdiff_pool.tile([P, 2 * D], F32, tag="pd")
                pd_t[m] = pd
                for k in range(2):
                    nc.tensor.matmul(
                        pd[:, k * D:(k + 1) * D],
                        lhsT=id_sb[:, 0:P],
                        rhs=xr_sb[:, m * D:(m + 1) * D],
                        start=True, stop=False)
                    nc.tensor.matmul(
                        pd[:, k * D:(k + 1) * D],
                        lhsT=id_sb[:, P:2 * P],
                        rhs=cand_t[m][:, k * D:(k + 1) * D],
                        start=False, stop=True)

            def stage_e(m):
                # ACT: dist2_k = sum((diff_k + eps)^2) -> resident tile
                pd = pd_t[m]
                sq = small_pool.tile([P, 2 * D], BF16, tag="sq")
                for k in range(2):
                    nc.scalar.activation(
                        out=sq[:, k * D:(k + 1) * D], in_=pd[:, k * D:(k + 1) * D],
                        func=mybir.ActivationFunctionType.Square,
                        bias=epsb[:], scale=1.0,
                        accum_out=d2_all[:, 2 * m + k:2 * m + k + 1])

            for m in range(M + 2):
                if 2 <= m <= M + 1:
                    stage_d(m - 2)
                    stage_e(m - 2)
                if m < M:
                    stage_a(m)
                if 1 <= m <= M:
                    stage_b(m - 1)
                    stage_c(m - 1)

            nc.sync.dma_start(d2_out[:], d2_all[:])
            nc.sync.dma_start(off_out[:], off_all[:])
    nc.compile()
    return nc


_CACHE = {}


def _built():
    if "nc" not in _CACHE:
        _CACHE["nc"] = build_bass(8)
    return _CACHE["nc"]


def make_in_maps(x):
    x = np.ascontiguousarray(np.asarray(x, dtype=np.float32))
    assert x.shape == (B, T, D)
    idm = np.eye(P, dtype=ml_dtypes.bfloat16)
    ident = np.ascontiguousarray(np.concatenate([idm, -idm], axis=1))
    in_maps = []
    for b in range(B):
        xb = x[b].astype(ml_dtypes.bfloat16)
        x8 = x[b].astype(ml_dtypes.float8_e4m3)
        # xT8[ki, ko, t] = x8[t, ko*128 + ki]
        xT8 = np.ascontiguousarray(
            x8.T.reshape(KC, P, T).transpose(1, 0, 2))
        in_maps.append({
            "xT8": xT8,
            "xrb": np.ascontiguousarray(
                xb.reshape(M, P, D).transpose(1, 0, 2)).reshape(P, M * D),
            "xgb": xb,
            "ident": ident,
        })
    return in_maps


def postprocess(results):
    """results: per-core dicts with d2 [128, 2M] and off [128, 2M].
    Row t = 128*m + p holds candidates at columns (2m, 2m+1)."""
    total = 0.0
    n = 0
    for r in results:
        d2 = r["d2"].astype(np.float64).reshape(P, M, 2)
        off = r["off"].astype(np.int64).reshape(P, M, 2)
        rowid = np.arange(P)[:, None, None] + 128 * np.arange(M)[None, :, None]
        d2 = np.where(off == rowid, np.inf, d2).min(axis=2)  # diag guard + min
        d = np.sqrt(d2)
        total += np.log(d + EPS).sum()
        n += d.size
    return np.float32(-(total / n))


def kernel(student_output):
    nc = _built()
    in_maps = make_in_maps(student_output)
    res = bass_utils.run_bass_kernel_spmd(nc, in_maps, core_ids=list(range(B)))
    return postprocess([res.results[b] for b in range(B)])


# revision 17
# speedup vs baseline: 1.0264x; 1.0044x over previous
"""KoLeo-loss kernel for Trainium2 (Bass/Tile), data-parallel over batch on 8 cores.

Input : student_output [8, 4096, 256] fp32
Output: scalar fp32 loss = -mean(log(||x - x_nn + 1e-8||_2 + 1e-8))
        where x_nn[b,t] = x[b, argmax_s <x[b,t], x[b,s]> (diag excluded)].

Per-core plan (core b handles batch b), "S7-fp8" scheme:
  - PE: gram matrix dots = x @ x.T in 32 m-tiles of [128, 4096] using
        fp8e4m3 inputs with DoubleRow perf mode: one matmul contracts
        the full K=256 (two 128-deep planes packed per PE cell) at 0.5
        cycles/row.  fp8 quantization only perturbs the *selection* of
        the argmax (ties flip to a near-equal neighbor); distances are
        computed from bf16 rows, so the loss error stays ~2e-4.
  - ACT: PSUM -> SBUF copies, downcast to bf16.
  - DVE: pairwise-max fold cascade 4096 -> 2048 -> ... -> 128 (2x mode
        in bf16), MAX8 on the 128-wide tail (top-1 is always the
        diagonal; top-2 is the NN dot), FIND_INDEX8 on the 2048-wide
        fold level.  The NN's true column is idx or idx+2048: both are
        evaluated and the row's distance is their min (flips ~0.6% of
        rows to a closer-but-lower-dot neighbor; ~2e-4 rel loss impact).
  - GPSIMD/DGE: one combined gather of both candidate rows (bf16).
  - PE: diff_k = x_t - cand_k via identity-matmul accumulation.
  - ACT: dist2_k = sum((diff_k + 1e-8)^2) via Square with accumulate.
  - host: diagonal guard + min over the 2 candidates, then
        loss = -mean(log(sqrt(dist2) + 1e-8)) in f64, over all 8 cores.
"""

import numpy as np
import ml_dtypes

import concourse.bass as bass
import concourse.tile as tile
from concourse import bacc, mybir
from concourse import bass_utils

F32 = mybir.dt.float32
BF16 = mybir.dt.bfloat16
FP8 = mybir.dt.float8e4
U32 = mybir.dt.uint32

B, T, D = 8, 4096, 256
P = 128                  # partitions
M = T // P               # 32 m-tiles
KC = D // P              # 2 contraction planes (DoubleRow)
EPS = 1e-8


def build_bass(num_devices=8):
    nc = bacc.Bacc("TRN2", target_bir_lowering=False, debug=False,
                   num_devices=num_devices)
    xT8 = nc.dram_tensor("xT8", [P, KC, T], FP8, kind="ExternalInput")
    xrb = nc.dram_tensor("xrb", [P, M * D], BF16, kind="ExternalInput")
    xgb = nc.dram_tensor("xgb", [T, D], BF16, kind="ExternalInput")
    ident = nc.dram_tensor("ident", [P, 2 * P], BF16, kind="ExternalInput")
    d2_out = nc.dram_tensor("d2", [P, 2 * M], F32, kind="ExternalOutput")
    off_out = nc.dram_tensor("off", [P, 2 * M], U32, kind="ExternalOutput")

    with tile.TileContext(nc) as tc:
        with (
            tc.tile_pool(name="const", bufs=1) as const_pool,
            tc.tile_pool(name="dots", bufs=2) as dots_pool,
            tc.tile_pool(name="w2048", bufs=2) as w2048_pool,
            tc.tile_pool(name="cands", bufs=3) as cands_pool,
            tc.tile_pool(name="pdots", bufs=3, space="PSUM") as pdots_pool,
            tc.tile_pool(name="pdiff", bufs=2, space="PSUM") as pdiff_pool,
            tc.tile_pool(name="small", bufs=4) as small_pool,
            tc.tile_pool(name="res", bufs=1) as res_pool,
        ):
            # resident inputs.  xT8 gates the first matmuls, so it gets the
            # sync DMA queue to itself; xr/ident (needed ~2 iterations in)
            # load in parallel on the scalar queue.
            xT8_sb = const_pool.tile([P, KC, T], FP8, tag="xT8")
            nc.sync.dma_start(xT8_sb[:], xT8[:])
            xr_sb = const_pool.tile([P, M * D], BF16, tag="xr")
            nc.scalar.dma_start(xr_sb[:], xrb[:])
            id_sb = const_pool.tile([P, 2 * P], BF16, tag="ident")
            nc.scalar.dma_start(id_sb[:], ident[:])
            epsb = const_pool.tile([P, 1], F32, tag="epsb")
            nc.vector.memset(epsb[:], EPS)

            d2_all = res_pool.tile([P, 2 * M], F32, tag="d2")
            off_all = res_pool.tile([P, 2 * M], U32, tag="off")

            dots_t = [None] * M   # bf16 dots tiles
            cand_t = [None] * M   # gathered candidate rows
            pd_t = [None] * M     # diff PSUM tiles

            def stage_a(m):
                # PE: dots in 4 PSUM quarters (2 DoubleRow mms each);
                # ACT: copy to bf16
                dots = dots_pool.tile([P, T], BF16, tag="dots")
                dots_t[m] = dots
                lhsT = xT8_sb[:, :, m * P:(m + 1) * P]
                for q in range(4):
                    ps = pdots_pool.tile([P, 1024], F32, tag="ps")
                    for jj in range(2):
                        j0 = q * 1024 + jj * 512
                        nc.tensor.matmul(
                            ps[:, jj * 512:(jj + 1) * 512],
                            lhsT=lhsT,
                            rhs=xT8_sb[:, :, j0:j0 + 512],
                            start=True, stop=True,
                            perf_mode=mybir.MatmulPerfMode.DoubleRow)
                    nc.scalar.copy(dots[:, q * 1024:(q + 1) * 1024], ps[:])

            def stage_b(m):
                # DVE: fold-max cascade + MAX8 + FIND_INDEX8@2048 + offsets
                dots = dots_t[m]
                w2048 = w2048_pool.tile([P, 2048], BF16, tag="w2048")
                nc.vector.tensor_tensor(
                    out=w2048[:], in0=dots[:, 0:2048], in1=dots[:, 2048:4096],
                    op=mybir.AluOpType.max)
                prev = w2048
                width = 1024
                folds = {}
                while width >= 128:
                    wt = small_pool.tile([P, width], BF16, tag=f"w{width}")
                    nc.vector.tensor_tensor(
                        out=wt[:], in0=prev[:, 0:width], in1=prev[:, width:2 * width],
                        op=mybir.AluOpType.max)
                    folds[width] = wt
                    prev = wt
                    width //= 2
                top8 = small_pool.tile([P, 8], BF16, tag="top8")
                nc.vector.max(out=top8[:], in_=folds[128][:])
                idx8 = small_pool.tile([P, 8], U32, tag="idx8")
                nc.vector.max_index(out=idx8[:], in_max=top8[:], in_values=w2048[:])

                # candidate columns: idx2 (clamped) and idx2 + 2048,
                # written straight into the resident output tile
                offs = off_all[:, 2 * m:2 * m + 2]
                nc.vector.tensor_scalar(
                    out=offs[:, 0:1], in0=idx8[:, 1:2], scalar1=2047,
                    scalar2=None, op0=mybir.AluOpType.min)
                nc.vector.tensor_scalar(
                    out=offs[:, 1:2], in0=offs[:, 0:1], scalar1=2048,
                    scalar2=None, op0=mybir.AluOpType.add)

            def stage_c(m):
                # GPSIMD/DGE: gather the two candidate rows (one indirect
                # DMA per offset column; a combined [P,2]-offset gather
                # returns garbage for the second slot on this runtime)
                cands = cands_pool.tile([P, 2 * D], BF16, tag="cands")
                cand_t[m] = cands
                for k in range(2):
                    nc.gpsimd.indirect_dma_start(
                        out=cands[:, k * D:(k + 1) * D], out_offset=None,
                        in_=xgb[:],
                        in_offset=bass.IndirectOffsetOnAxis(
                            ap=off_all[:, 2 * m + k:2 * m + k + 1], axis=0))

            def stage_d(m):
                # PE: diff_k = x_t - cand_k via identity matmuls
                pd = pdiff_pool.tile([P, 2 * D], F32, tag="pd")
                pd_t[m] = pd
                for k in range(2):
                    nc.tensor.matmul(
                        pd[:, k * D:(k + 1) * D],
                        lhsT=id_sb[:, 0:P],
                        rhs=xr_sb[:, m * D:(m + 1) * D],
                        start=True, stop=False)
                    nc.tensor.matmul(
                        pd[:, k * D:(k + 1) * D],
                        lhsT=id_sb[:, P:2 * P],
                        rhs=cand_t[m][:, k * D:(k + 1) * D],
                        start=False, stop=True)

            def stage_e(m):
                # ACT: dist2_k = sum((diff_k + eps)^2) -> resident tile
                pd = pd_t[m]
                sq = small_pool.tile([P, 2 * D], BF16, tag="sq")
                for k in range(2):
                    nc.scalar.activation(
                        out=sq[:, k * D:(k + 1) * D], in_=pd[:, k * D:(k + 1) * D],
                        func=mybir.ActivationFunctionType.Square,
                        bias=epsb[:], scale=1.0,
                        accum_out=d2_all[:, 2 * m + k:2 * m + k + 1])

            for m in range(M + 2):
                if 2 <= m <= M + 1:
                    stage_d(m - 2)
                    stage_e(m - 2)
                if m < M:
                    stage_a(m)
                if 1 <= m <= M:
                    stage_b(m - 1)
                    stage_c(m - 1)

            nc.sync.dma_start(d2_out[:], d2_all[:])
            nc.sync.dma_start(off_out[:], off_all[:])
    nc.compile()
    return nc


_CACHE = {}


def _built():
    if "nc" not in _CACHE:
        _CACHE["nc"] = build_bass(8)
    return _CACHE["nc"]


def make_in_maps(x):
    x = np.ascontiguousarray(np.asarray(x, dtype=np.float32))
    assert x.shape == (B, T, D)
    idm = np.eye(P, dtype=ml_dtypes.bfloat16)
    ident = np.ascontiguousarray(np.concatenate([idm, -idm], axis=1))
    in_maps = []
    for b in range(B):
        xb = x[b].astype(ml_dtypes.bfloat16)
        x8 = x[b].astype(ml_dtypes.float8_e4m3)
        # xT8[ki, ko, t] = x8[t, ko*128 + ki]
        xT8 = np.ascontiguousarray(
            x8.T.reshape(KC, P, T).transpose(1, 0, 2))
        in_maps.append({
            "xT8": xT8,
            "xrb": np.ascontiguousarray(
                xb.reshape(M, P, D).transpose(1, 0, 2)).reshape(P, M * D),
            "xgb": xb,
            "ident": ident,
        })
    return in_maps


def postprocess(results):
    """results: per-core dicts with d2 [128, 2M] and off [128, 2M].
    Row t = 128*m + p holds candidates at columns (2m, 2m+1)."""
    total = 0.0
    n = 0
    for r in results:
        d2 = r["d2"].astype(np.float64).reshape(P, M, 2)
        off = r["off"].astype(np.int64).reshape(P, M, 2)
        rowid = np.arange(P)[:, None, None] + 128 * np.arange(M)[None, :, None]
        d2 = np.where(off == rowid, np.inf, d2).min(axis=2)  # diag guard + min
        d = np.sqrt(d2)
        total += np.log(d + EPS).sum()
        n += d.size
    return np.float32(-(total / n))


def kernel(student_output):
    nc = _built()
    in_maps = make_in_maps(student_output)
    res = bass_utils.run_bass_kernel_spmd(nc, in_maps, core_ids=list(range(B)))
    return postprocess([res.results[b] for b in range(B)])


# revision 19
# speedup vs baseline: 1.0285x; 1.0020x over previous
"""KoLeo-loss kernel for Trainium2 (Bass/Tile), data-parallel over batch on 8 cores.

Input : student_output [8, 4096, 256] fp32
Output: scalar fp32 loss = -mean(log(||x - x_nn + 1e-8||_2 + 1e-8))
        where x_nn[b,t] = x[b, argmax_s <x[b,t], x[b,s]> (diag excluded)].

Per-core plan (core b handles batch b), "S7-fp8" scheme:
  - PE: gram matrix dots = x @ x.T in 32 m-tiles of [128, 4096] using
        fp8e4m3 inputs with DoubleRow perf mode: one matmul contracts
        the full K=256 (two 128-deep planes packed per PE cell) at 0.5
        cycles/row.  fp8 quantization only perturbs the *selection* of
        the argmax (ties flip to a near-equal neighbor); distances are
        computed from bf16 rows, so the loss error stays ~2e-4.
  - ACT: PSUM -> SBUF copies, downcast to bf16.
  - DVE: pairwise-max fold cascade 4096 -> 2048 -> ... -> 128 (2x mode
        in bf16), MAX8 on the 128-wide tail (top-1 is always the
        diagonal; top-2 is the NN dot), FIND_INDEX8 on the 2048-wide
        fold level.  The NN's true column is idx or idx+2048: both are
        evaluated and the row's distance is their min (flips ~0.6% of
        rows to a closer-but-lower-dot neighbor; ~2e-4 rel loss impact).
  - GPSIMD/DGE: one combined gather of both candidate rows (bf16).
  - PE: diff_k = x_t - cand_k via identity-matmul accumulation.
  - ACT: dist2_k = sum((diff_k + 1e-8)^2) via Square with accumulate.
  - host: diagonal guard + min over the 2 candidates, then
        loss = -mean(log(sqrt(dist2) + 1e-8)) in f64, over all 8 cores.
"""

import numpy as np
import ml_dtypes

import concourse.bass as bass
import concourse.tile as tile
from concourse import bacc, mybir
from concourse import bass_utils

F32 = mybir.dt.float32
BF16 = mybir.dt.bfloat16
FP8 = mybir.dt.float8e4
U32 = mybir.dt.uint32

B, T, D = 8, 4096, 256
P = 128                  # partitions
M = T // P               # 32 m-tiles
KC = D // P              # 2 contraction planes (DoubleRow)
EPS = 1e-8


def build_bass(num_devices=8):
    nc = bacc.Bacc("TRN2", target_bir_lowering=False, debug=False,
                   num_devices=num_devices)
    xT8 = nc.dram_tensor("xT8", [P, KC, T], FP8, kind="ExternalInput")
    xrb = nc.dram_tensor("xrb", [P, M * D], BF16, kind="ExternalInput")
    xgb = nc.dram_tensor("xgb", [T, D], BF16, kind="ExternalInput")
    ident = nc.dram_tensor("ident", [P, 2 * P], BF16, kind="ExternalInput")
    d2_out = nc.dram_tensor("d2", [P, 2 * M], F32, kind="ExternalOutput")
    off_out = nc.dram_tensor("off", [P, 2 * M], U32, kind="ExternalOutput")

    with tile.TileContext(nc) as tc:
        with (
            tc.tile_pool(name="const", bufs=1) as const_pool,
            tc.tile_pool(name="dots", bufs=2) as dots_pool,
            tc.tile_pool(name="w2048", bufs=2) as w2048_pool,
            tc.tile_pool(name="cands", bufs=3) as cands_pool,
            tc.tile_pool(name="pdots", bufs=3, space="PSUM") as pdots_pool,
            tc.tile_pool(name="pdiff", bufs=2, space="PSUM") as pdiff_pool,
            tc.tile_pool(name="small", bufs=4) as small_pool,
            tc.tile_pool(name="res", bufs=1) as res_pool,
        ):
            # resident inputs.  xT8 gates the first matmuls; xr/ident are
            # not needed until stage_d(0) (~2 iterations in), so their DMA
            # triggers are issued inside stage_a(0) behind the first copy,
            # leaving the full HBM bandwidth to xT8 at t=0.
            xT8_sb = const_pool.tile([P, KC, T], FP8, tag="xT8")
            nc.sync.dma_start(xT8_sb[:], xT8[:])
            xr_sb = const_pool.tile([P, M * D], BF16, tag="xr")
            id_sb = const_pool.tile([P, 2 * P], BF16, tag="ident")
            epsb = const_pool.tile([P, 1], F32, tag="epsb")
            nc.vector.memset(epsb[:], EPS)

            d2_all = res_pool.tile([P, 2 * M], F32, tag="d2")
            off_all = res_pool.tile([P, 2 * M], U32, tag="off")

            dots_t = [None] * M   # bf16 dots tiles
            cand_t = [None] * M   # gathered candidate rows
            pd_t = [None] * M     # diff PSUM tiles

            def stage_a(m):
                # PE: dots in 4 PSUM quarters (2 DoubleRow mms each);
                # ACT: copy to bf16
                dots = dots_pool.tile([P, T], BF16, tag="dots")
                dots_t[m] = dots
                lhsT = xT8_sb[:, :, m * P:(m + 1) * P]
                for q in range(4):
                    ps = pdots_pool.tile([P, 1024], F32, tag="ps")
                    for jj in range(2):
                        j0 = q * 1024 + jj * 512
                        nc.tensor.matmul(
                            ps[:, jj * 512:(jj + 1) * 512],
                            lhsT=lhsT,
                            rhs=xT8_sb[:, :, j0:j0 + 512],
                            start=True, stop=True,
                            perf_mode=mybir.MatmulPerfMode.DoubleRow)
                    nc.scalar.copy(dots[:, q * 1024:(q + 1) * 1024], ps[:])
                    if m == 0 and q == 0:
                        # deferred input loads (see const setup above)
                        nc.scalar.dma_start(xr_sb[:], xrb[:])
                        nc.scalar.dma_start(id_sb[:], ident[:])

            def stage_b(m):
                # DVE: fold-max cascade + MAX8 + FIND_INDEX8@2048 + offsets
                dots = dots_t[m]
                w2048 = w2048_pool.tile([P, 2048], BF16, tag="w2048")
                nc.vector.tensor_tensor(
                    out=w2048[:], in0=dots[:, 0:2048], in1=dots[:, 2048:4096],
                    op=mybir.AluOpType.max)
                prev = w2048
                width = 1024
                folds = {}
                while width >= 128:
                    wt = small_pool.tile([P, width], BF16, tag=f"w{width}")
                    nc.vector.tensor_tensor(
                        out=wt[:], in0=prev[:, 0:width], in1=prev[:, width:2 * width],
                        op=mybir.AluOpType.max)
                    folds[width] = wt
                    prev = wt
                    width //= 2
                top8 = small_pool.tile([P, 8], BF16, tag="top8")
                nc.vector.max(out=top8[:], in_=folds[128][:])
                idx8 = small_pool.tile([P, 8], U32, tag="idx8")
                nc.vector.max_index(out=idx8[:], in_max=top8[:], in_values=w2048[:])

                # candidate columns: idx2 (clamped) and idx2 + 2048,
                # written straight into the resident output tile
                offs = off_all[:, 2 * m:2 * m + 2]
                nc.vector.tensor_scalar(
                    out=offs[:, 0:1], in0=idx8[:, 1:2], scalar1=2047,
                    scalar2=None, op0=mybir.AluOpType.min)
                nc.vector.tensor_scalar(
                    out=offs[:, 1:2], in0=offs[:, 0:1], scalar1=2048,
                    scalar2=None, op0=mybir.AluOpType.add)

            def stage_c(m):
                # GPSIMD/DGE: gather the two candidate rows (one indirect
                # DMA per offset column; a combined [P,2]-offset gather
                # returns garbage for the second slot on this runtime)
                cands = cands_pool.tile([P, 2 * D], BF16, tag="cands")
                cand_t[m] = cands
                for k in range(2):
                    nc.gpsimd.indirect_dma_start(
                        out=cands[:, k * D:(k + 1) * D], out_offset=None,
                        in_=xgb[:],
                        in_offset=bass.IndirectOffsetOnAxis(
                            ap=off_all[:, 2 * m + k:2 * m + k + 1], axis=0))

            def stage_d(m):
                # PE: diff_k = x_t - cand_k via identity matmuls
                pd = pdiff_pool.tile([P, 2 * D], F32, tag="pd")
                pd_t[m] = pd
                for k in range(2):
                    nc.tensor.matmul(
                        pd[:, k * D:(k + 1) * D],
                        lhsT=id_sb[:, 0:P],
                        rhs=xr_sb[:, m * D:(m + 1) * D],
                        start=True, stop=False)
                    nc.tensor.matmul(
                        pd[:, k * D:(k + 1) * D],
                        lhsT=id_sb[:, P:2 * P],
                        rhs=cand_t[m][:, k * D:(k + 1) * D],
                        start=False, stop=True)

            def stage_e(m):
                # ACT: dist2_k = sum((diff_k + eps)^2) -> resident tile
                pd = pd_t[m]
                sq = small_pool.tile([P, 2 * D], BF16, tag="sq")
                for k in range(2):
                    nc.scalar.activation(
                        out=sq[:, k * D:(k + 1) * D], in_=pd[:, k * D:(k + 1) * D],
                        func=mybir.ActivationFunctionType.Square,
                        bias=epsb[:], scale=1.0,
                        accum_out=d2_all[:, 2 * m + k:2 * m + k + 1])

            for m in range(M + 2):
                if 2 <= m <= M + 1:
                    stage_d(m - 2)
                    stage_e(m - 2)
                if m < M:
                    stage_a(m)
                if 1 <= m <= M:
                    stage_b(m - 1)
                    stage_c(m - 1)

            nc.sync.dma_start(d2_out[:], d2_all[:])
            nc.sync.dma_start(off_out[:], off_all[:])
    nc.compile()
    return nc


_CACHE = {}


def _built():
    if "nc" not in _CACHE:
        _CACHE["nc"] = build_bass(8)
    return _CACHE["nc"]


def make_in_maps(x):
    x = np.ascontiguousarray(np.asarray(x, dtype=np.float32))
    assert x.shape == (B, T, D)
    idm = np.eye(P, dtype=ml_dtypes.bfloat16)
    ident = np.ascontiguousarray(np.concatenate([idm, -idm], axis=1))
    in_maps = []
    for b in range(B):
        xb = x[b].astype(ml_dtypes.bfloat16)
        x8 = x[b].astype(ml_dtypes.float8_e4m3)
        # xT8[ki, ko, t] = x8[t, ko*128 + ki]
        xT8 = np.ascontiguousarray(
            x8.T.reshape(KC, P, T).transpose(1, 0, 2))
        in_maps.append({
            "xT8": xT8,
            "xrb": np.ascontiguousarray(
                xb.reshape(M, P, D).transpose(1, 0, 2)).reshape(P, M * D),
            "xgb": xb,
            "ident": ident,
        })
    return in_maps


def postprocess(results):
    """results: per-core dicts with d2 [128, 2M] and off [128, 2M].
    Row t = 128*m + p holds candidates at columns (2m, 2m+1)."""
    total = 0.0
    n = 0
    for r in results:
        d2 = r["d2"].astype(np.float64).reshape(P, M, 2)
        off = r["off"].astype(np.int64).reshape(P, M, 2)
        rowid = np.arange(P)[:, None, None] + 128 * np.arange(M)[None, :, None]
        d2 = np.where(off == rowid, np.inf, d2).min(axis=2)  # diag guard + min
        d = np.sqrt(d2)
        total += np.log(d + EPS).sum()
        n += d.size
    return np.float32(-(total / n))


def kernel(student_output):
    nc = _built()
    in_maps = make_in_maps(student_output)
    res = bass_utils.run_bass_kernel_spmd(nc, in_maps, core_ids=list(range(B)))
    return postprocess([res.results[b] for b in range(B)])
